# revision 1
# baseline (speedup 1.0000x reference)
"""Trainium2 Bass kernel for MultiHeadedAttention with learned per-key-position scaling.

Sharding over 8 NeuronCores: batch(2) x q-half(2) x head-half(2).
Each core: its batch's full keys/values, a 1024-row query slice, 6 heads.

Scores are computed transposed ([kpos, q]) so that:
  - the per-key-position divisor delta folds into the exp's per-partition scale,
  - the softmax denominator Z comes from a ones-column appended to V,
  - the P@V matmul runs "flipped": P chunks are the stationary operand and
    V-hat (V plus the ones column) streams, so each of the 16 key-chunk
    accumulation steps streams only 65 columns instead of replaying all
    1024 query columns (PE cost is proportional to streamed columns).

The flipped P@V produces x as [q, dv]; the softmax normalization 1/Z then
becomes a per-partition scalar multiply fused into the PSUM evacuation, and
the [q, dh] -> [dh, q] layout change for the output projection is done by the
XBAR DMA transpose engine (off the compute engines entirely).

The V-projection bias is folded out of the device kernel: x = P@(V0 + 1*bv^T)
normalizes to x0/Z + bv, so the host adds bv @ Wo into the output bias.

Precision: the q/k path (projections + scores) runs in float32r (PE's fast
rounded-fp32), value, attention probabilities and the output projection run
in bf16. delta is computed in fp32 from each core's own query slice and
exchanged between q-half partner cores with a tiny AllGather.
The host pre-transposes activations into [d_model, seq] layout so all device
DMAs are plain contiguous loads.

Host combines per-core partial outputs (sum over head-halves + bo').
"""

import sys

for _p in ("/opt/trn_rl_repo",):
    if _p not in sys.path:
        sys.path.insert(0, _p)

import numpy as np
import ml_dtypes

BF16 = ml_dtypes.bfloat16

B, S, D, H, DK = 2, 2048, 768, 12, 64
NCORES = 8
SQ = S // 2          # query rows per core
HH = H // 2          # heads per core
DH = HH * DK         # 384 head dims per core

_cache = {}


def _build(s=S, sq=SQ, hh=HH, d=D, dk=DK, n_qh=2, dbg=False, MASK_NG=8):
    import concourse.bass as bass
    import concourse.mybir as mybir
    import concourse.tile as tile
    from concourse import bacc

    f32 = mybir.dt.float32
    f32r = mybir.dt.float32r
    bf = mybir.dt.bfloat16
    Exp = mybir.ActivationFunctionType.Exp
    mult = mybir.AluOpType.mult
    add = mybir.AluOpType.add
    amin = mybir.AluOpType.min
    amax = mybir.AluOpType.max

    dh = hh * dk
    KC = s // 128        # key-position chunks
    C6 = d // 128        # d_model chunks
    C3 = dh // 128       # output-dim chunks per core
    NQ = sq // 512       # 512-wide q column blocks (scores)
    QC = sq // 128       # q row chunks
    BW = 256             # streaming block width (projection inputs)
    NBK = s // BW        # key/value stream blocks
    NBQ = sq // BW       # query-slice stream blocks
    KCL = BW // 128      # kpos chunks per stream block

    # AllGather partners: cores sharing (batch, head-half), differing in
    # q-half; q-half 0 listed first so the gather lands in global key order.
    groups = [[b * 4 + hf, b * 4 + 2 + hf] for b in range(2) for hf in range(2)]
    if n_qh == 1:
        groups = None

    nc = bacc.Bacc("TRN2", target_bir_lowering=False, debug=False, num_devices=NCORES)

    t = {}
    t["qqT"] = nc.dram_tensor("qqT", [d, sq], f32r, kind="ExternalInput").ap()
    t["kT"] = nc.dram_tensor("kT", [d, s], f32r, kind="ExternalInput").ap()
    t["vT"] = nc.dram_tensor("vT", [d, s], bf, kind="ExternalInput").ap()
    t["maskT"] = nc.dram_tensor("maskT", [s, sq], bf, kind="ExternalInput").ap()
    t["wq"] = nc.dram_tensor("wq", [d, dh], f32r, kind="ExternalInput").ap()
    t["wk"] = nc.dram_tensor("wk", [d, dh], f32r, kind="ExternalInput").ap()
    t["wv"] = nc.dram_tensor("wv", [d, dh], bf, kind="ExternalInput").ap()
    t["wo"] = nc.dram_tensor("wo", [dh, d], bf, kind="ExternalInput").ap()
    t["wd"] = nc.dram_tensor("wd", [d, 1], f32, kind="ExternalInput").ap()
    t["bq"] = nc.dram_tensor("bq", [dh], f32, kind="ExternalInput").ap()
    t["bk"] = nc.dram_tensor("bk", [dh], f32, kind="ExternalInput").ap()
    t["bd"] = nc.dram_tensor("bd", [1], f32, kind="ExternalInput").ap()
    t["yp"] = nc.dram_tensor("yp", [sq, d], bf, kind="ExternalOutput").ap()
    if dbg:
        t["dxh"] = nc.dram_tensor("dxh", [128, sq // 128, hh, dk], bf, kind="ExternalOutput").ap()
        t["dxT"] = nc.dram_tensor("dxT", [128, (hh * dk) // 128, sq], bf, kind="ExternalOutput").ap()
        t["dps"] = nc.dram_tensor("dps", [128, s // 128, sq], bf, kind="ExternalOutput").ap()
        t["drz"] = nc.dram_tensor("drz", [128, hh, sq // 128], f32, kind="ExternalOutput").ap()

    # [d, *] tensors viewed as [128, C6, *] partition tiles
    def dview(ap):
        return ap.rearrange("(c p) s -> p c s", p=128)

    def bcast(ap, n):
        # broadcast a 1-D DRAM vector across n partitions
        return bass.AP(tensor=ap.tensor, offset=ap.offset, ap=[[0, n]] + list(ap.ap))

    with tile.TileContext(nc) as tc:
        with (
            tc.tile_pool(name="persist", bufs=1) as P,
            tc.tile_pool(name="pj", bufs=2, space="PSUM") as PJ,
            tc.tile_pool(name="xpp", bufs=3, space="PSUM") as XPP,
            tc.tile_pool(name="xv", bufs=1, space="PSUM") as XV,
            tc.tile_pool(name="work", bufs=1) as W,
            tc.tile_pool(name="work2", bufs=4) as W2,
            tc.tile_pool(name="load", bufs=4) as L,
            tc.tile_pool(name="loadfr", bufs=2) as LF,
            tc.tile_pool(name="dram", bufs=2, space="DRAM") as DR,
        ):
            maskT = P.tile([128, KC, sq], bf)
            vsb = P.tile([128, KC, hh, dk + 1], bf)
            # rolling store of masked attention probabilities: 16 live chunks
            # per head + 8 slots of slack so a head's P@V groups can drain
            # during the NEXT head's score/exp/mask stream without colliding
            PSS = KC + 8
            psS = P.tile([128, PSS, sq], bf)
            qTh = P.tile([128, C3, sq], f32r)    # head pairs packed on partitions
            kTh = P.tile([128, C3, s], f32r)
            xh = P.tile([128, QC, hh, dk], bf)   # attention out, [q, head, dv]
            xT = P.tile([128, C3, sq], bf)       # transposed for the out-proj
            wq_sb = P.tile([128, C6, dh], f32r)
            wk_sb = P.tile([128, C6, dh], f32r)
            wv_sb = P.tile([128, C6, dh], bf)
            wo_sb = P.tile([128, C3, d], bf)
            wd_sb = P.tile([128, C6, 1], f32)
            bqc = P.tile([128, C3], f32)
            bkc = P.tile([128, C3], f32)
            bdb = P.tile([128, 1], f32)
            rdcol = P.tile([128, KC], f32)

            # warm the ACT exp table while DMAs stream
            dummy = W.tile([1, 2], f32, tag="dummy")
            nc.vector.memset(dummy, 0.0)
            nc.scalar.activation(dummy, dummy, Exp, scale=1.0)

            # small-vector loads dispatch from the (idle) ACT sequencer so
            # the SP sequencer's 650ns-per-dispatch serialization starts with
            # the critical qq/wq stream instead
            nc.scalar.dma_start(wd_sb, dview(t["wd"]))
            nc.gpsimd.dma_start(bdb, bcast(t["bd"], 128))
            nc.scalar.dma_start(bqc, t["bq"].rearrange("(c p) -> p c", p=128))
            nc.vector.memset(vsb[:, :, :, dk : dk + 1], 1.0)

            def f32r_load(src_ap, blk):
                # direct f32r load (input tensors are declared f32r; the
                # PE rounds on read, verified on hardware)
                fr = L.tile([128, C6, BW], f32r, tag="ldf")
                nc.sync.dma_start(fr, src_ap[:, :, blk * BW : (blk + 1) * BW])
                return fr

            NG = MASK_NG

            def mask_g(g):
                nc.sync.dma_start(
                    maskT[:, g * (KC // NG) : (g + 1) * (KC // NG), :],
                    t["maskT"].rearrange("(kc p) q -> p kc q", p=128)[
                        :, g * (KC // NG) : (g + 1) * (KC // NG), :
                    ],
                )

            # stream emitters, interleaved with head-0 attention below
            def k_block(blk, pre=None):
                kfb = pre if pre is not None else f32r_load(dview(t["kT"]), blk)
                for m in range(C3):
                    kp = PJ.tile([128, BW], f32, tag="pj")
                    for c in range(C6):
                        nc.tensor.matmul(
                            kp,
                            lhsT=wk_sb[:, c, m * 128 : (m + 1) * 128],
                            rhs=kfb[:, c, :],
                            start=(c == 0),
                            stop=(c == C6 - 1),
                        )
                    nc.vector.tensor_scalar_add(
                        out=kTh[:, m, blk * BW : (blk + 1) * BW],
                        in0=kp,
                        scalar1=bkc[:, m : m + 1],
                    )


            # --- Q projection (+ local delta) over the query slice ---
            dps = XV.tile([128, sq // 128], f32, tag="vp")
            for blk in range(NBQ):
                qqb = f32r_load(dview(t["qqT"]), blk)
                qqf = qqb.bitcast(f32)
                if blk == 0:
                    # q/k weights ride behind the first query block
                    nc.sync.dma_start(wq_sb, dview(t["wq"]))
                    nc.sync.dma_start(wk_sb, dview(t["wk"]))
                    nc.scalar.dma_start(bkc, t["bk"].rearrange("(c p) -> p c", p=128))
                if blk == 1:
                    # k-block-0's load right after qq1: its compute interleaves
                    # into the Q-projection (PE is in-order), pulling the whole
                    # head-0 stream phase earlier
                    k0_tile = f32r_load(dview(t["kT"]), 0)
                if blk == 2:
                    k_block(0, pre=k0_tile)
                    k_done = 1
                for kcl in range(KCL):
                    for c in range(C6):
                        nc.tensor.matmul(
                            dps[:, blk * KCL + kcl : blk * KCL + kcl + 1],
                            lhsT=qqf[:, c, kcl * 128 : (kcl + 1) * 128],
                            rhs=wd_sb[:, c, :],
                            start=(c == 0),
                            stop=(c == C6 - 1),
                        )
                for m in range(C3):
                    qp = PJ.tile([128, BW], f32, tag="pj")
                    for c in range(C6):
                        nc.tensor.matmul(
                            qp,
                            lhsT=wq_sb[:, c, m * 128 : (m + 1) * 128],
                            rhs=qqb[:, c, :],
                            start=(c == 0),
                            stop=(c == C6 - 1),
                        )
                    nc.vector.tensor_scalar_add(
                        out=qTh[:, m, blk * BW : (blk + 1) * BW],
                        in0=qp,
                        scalar1=bqc[:, m : m + 1],
                    )

            # local delta -> recip -> exchange with q-half partner
            dloc = W2.tile([128, sq // 128], f32, tag="dloc")
            nc.vector.tensor_scalar(
                out=dloc, in0=dps, scalar1=bdb, scalar2=0.0, op0=add, op1=amax
            )
            nc.vector.tensor_scalar(
                out=dloc, in0=dloc, scalar1=8.0, scalar2=1.0, op0=amin, op1=add
            )
            nc.vector.reciprocal(dloc, dloc)
            # (p, kc)-major DRAM layout keeps every exchange hop contiguous
            # per partition (tiny descriptor counts), so the chain slips
            # through DMA-pipe gaps instead of queueing behind bulk streams
            gin = DR.tile([sq], f32)
            nc.sync.dma_start(gin.rearrange("(p kcl) -> p kcl", p=128), dloc)
            gout = DR.tile([s], f32)
            if groups is not None:
                nc.gpsimd.collective_compute(
                    "AllGather",
                    mybir.AluOpType.bypass,
                    replica_groups=groups,
                    ins=[gin.opt()],
                    outs=[gout.opt()],
                )
            else:
                nc.sync.dma_start(
                    gout.rearrange("(qh rest) -> qh rest", qh=2),
                    bass.AP(
                        tensor=gin.tensor,
                        offset=gin.offset,
                        ap=[[0, 2]] + list(gin.ap),
                    ),
                )
            nc.sync.dma_start(
                rdcol.rearrange("p (qh kcl) -> p qh kcl", qh=s // sq),
                gout.rearrange("(qh p kcl) -> p qh kcl", qh=s // sq, p=128),
            )

            nc.sync.dma_start(wv_sb, dview(t["wv"]))

            def v_block(blk):
                vT = LF.tile([128, C6, BW], bf, tag="vb")
                nc.sync.dma_start(
                    vT, dview(t["vT"])[:, :, blk * BW : (blk + 1) * BW]
                )
                for kcl in range(KCL):
                    kc = blk * KCL + kcl
                    vp = XV.tile([128, dh], f32, tag="vp")
                    for c in range(C6):
                        nc.tensor.matmul(
                            vp,
                            lhsT=vT[:, c, kcl * 128 : (kcl + 1) * 128],
                            rhs=wv_sb[:, c, :],
                            start=(c == 0),
                            stop=(c == C6 - 1),
                        )
                    nc.vector.tensor_copy(
                        vsb[:, kc, :, 0:dk],
                        vp.rearrange("p (h e) -> p h e", h=hh),
                    )

            # --- attention; streams interleave with head 0 ---
            kc_per_blk = BW // 128
            LOOKAHEAD = 1
            v_block(0)
            v_done = 1
            # hold mask-g0's pipe entry past the (now tiny) delta-chain hops
            # that gate the first exp; its consumer trails by a full head
            with tc.tile_wait_until(0.026):
                mask_g(0)
            m_done = 1

            def slot(hd, kc):
                return (KC * hd + kc) % PSS

            def pv_drain(hd, qc):
                # one flipped-P@V accumulation group (own PSUM bank), fused
                # normalize on evacuation, and the pair transpose once the
                # odd head of a pair is drained
                xq = XPP.tile([128, 512], f32, tag="xps")
                for kc in range(KC):
                    nc.tensor.matmul(
                        xq[:, 0 : dk + 1],
                        lhsT=psS[:, slot(hd, kc), qc * 128 : (qc + 1) * 128],
                        rhs=vsb[:, kc, hd, :],
                        start=(kc == 0),
                        stop=(kc == KC - 1),
                    )
                rz = W2.tile([128, 1], f32, tag="rz")
                nc.vector.reciprocal(rz, xq[:, dk : dk + 1])
                if dbg:
                    nc.sync.dma_start(t["drz"][:, hd, qc : qc + 1], rz)
                nc.vector.tensor_scalar_mul(
                    out=xh[:, qc, hd, :],
                    in0=xq[:, 0:dk],
                    scalar1=rz,
                )
                if hd % 2 == 1:
                    # ACT-queue dispatch only in the epilogue (ACT idle); the
                    # mid-window pairs must not steal ACT.SEQ slots from exps
                    eng = nc.scalar if (hd == hh - 1 and qc % 2 == 1) else nc.sync
                    eng.dma_start(
                        xT[:, hd // 2, qc * 128 : (qc + 1) * 128],
                        xh[:, qc, hd - 1 : hd + 1, :],
                        transpose=True,
                    )

            for h in range(hh):
                hoff = (h % 2) * 64
                for kc in range(KC):
                    sps = PJ.tile([128, sq], f32, tag="pj")
                    for nn in range(NQ):
                        nc.tensor.matmul(
                            sps[:, nn * 512 : (nn + 1) * 512],
                            lhsT=kTh[
                                hoff : hoff + 64, h // 2, kc * 128 : (kc + 1) * 128
                            ],
                            rhs=qTh[
                                hoff : hoff + 64, h // 2, nn * 512 : (nn + 1) * 512
                            ],
                            start=True,
                            stop=True,
                        )
                    psb = psS[:, slot(h, kc), :]
                    nc.scalar.activation(psb, sps, Exp, scale=rdcol[:, kc : kc + 1])
                    nc.vector.tensor_tensor(
                        out=psb, in0=psb, in1=maskT[:, kc, :], op=mult
                    )
                    if dbg and h == 1:
                        nc.sync.dma_start(t["dps"][:, kc, :], psb)
                    if h > 0 and kc < QC:
                        # drain the previous head's P@V while this head's
                        # scores/exp/mask stream keeps ACT and PE busy
                        pv_drain(h - 1, kc)
                    if h == 0:
                        j = kc // kc_per_blk + LOOKAHEAD
                        if kc % kc_per_blk == 0:
                            if j < NBK:
                                k_block(j); k_done += 1
                            gsz = KC // NG
                            while m_done < NG and m_done <= (kc + kc_per_blk + gsz - 1) // gsz:
                                mask_g(m_done); m_done += 1
                        else:
                            if j < NBK:
                                v_block(j); v_done += 1
                        if kc == KC - 1:
                            while k_done < NBK:
                                k_block(k_done); k_done += 1
                            while v_done < NBK:
                                v_block(v_done); v_done += 1
                            nc.sync.dma_start(
                                wo_sb,
                                t["wo"].rearrange("(c p) m -> p c m", p=128),
                            )

            # --- epilogue: drain the last head per q-chunk, pipelined with
            # its pair transpose and the output projection (which starts its
            # accumulation with that freshest chunk: c order [C3-1, 0, 1])
            for qc in range(QC):
                pv_drain(hh - 1, qc)
                yps = PJ.tile([128, d], f32, tag="pj")
                corder = [C3 - 1] + list(range(C3 - 1))
                for ci, c in enumerate(corder):
                    for col in range(0, d, 512):
                        ncol = min(512, d - col)
                        nc.tensor.matmul(
                            yps[:, col : col + ncol],
                            lhsT=xT[:, c, qc * 128 : (qc + 1) * 128],
                            rhs=wo_sb[:, c, col : col + ncol],
                            start=(ci == 0),
                            stop=(ci == C3 - 1),
                        )
                ysb = W2.tile([128, d], bf, tag="ysb")
                if qc % 2 == 0:
                    nc.scalar.copy(ysb, yps)
                else:
                    nc.vector.tensor_copy(ysb, yps)
                nc.sync.dma_start(t["yp"][qc * 128 : (qc + 1) * 128, :], ysb)
            if dbg:
                nc.sync.dma_start(t["dxT"], xT)

    nc.compile()
    return nc


def _in_maps(query, key, value, mask, Wq, bq, Wk, bk, Wv, bv, Wo, Wd, bd, sq=SQ, dh=DH):
    query = np.asarray(query, np.float32)
    key = np.asarray(key, np.float32)
    value = np.asarray(value, np.float32)
    mask = np.asarray(mask)
    qT = [np.ascontiguousarray(query[b].T) for b in range(B)]
    kT = [np.ascontiguousarray(key[b].T) for b in range(B)]
    vT = [np.ascontiguousarray(value[b].T).astype(BF16) for b in range(B)]
    wqf = np.ascontiguousarray(Wq, np.float32)
    wkf = np.ascontiguousarray(Wk, np.float32)
    wvb = np.ascontiguousarray(Wv).astype(BF16)
    wob = np.ascontiguousarray(Wo).astype(BF16)
    wdf = np.ascontiguousarray(Wd, np.float32)
    bqf = np.ascontiguousarray(bq, np.float32)
    bkf = np.ascontiguousarray(bk, np.float32)
    bdf = np.ascontiguousarray(bd, np.float32)

    maps = []
    for c in range(NCORES):
        b, qh, hf = c // 4, (c // 2) % 2, c % 2
        qs = slice(qh * sq, (qh + 1) * sq)
        hs = slice(hf * dh, (hf + 1) * dh)
        maps.append(
            {
                "qqT": np.ascontiguousarray(qT[b][:, qs]),
                "kT": kT[b],
                "vT": vT[b],
                "maskT": np.ascontiguousarray(mask[b, qs].T).astype(BF16),
                "wq": np.ascontiguousarray(wqf[:, hs]),
                "wk": np.ascontiguousarray(wkf[:, hs]),
                "wv": np.ascontiguousarray(wvb[:, hs]),
                "wo": np.ascontiguousarray(wob[hs, :]),
                "wd": wdf,
                "bq": np.ascontiguousarray(bqf[hs]),
                "bk": np.ascontiguousarray(bkf[hs]),
                "bd": bdf,
            }
        )
    return maps


def kernel(query, key, value, mask, Wq, bq, Wk, bk, Wv, bv, Wo, bo, Wd, bd):
    from concourse.bass_utils import run_bass_kernel_spmd

    if "nc" not in _cache:
        _cache["nc"] = _build()
    nc = _cache["nc"]

    maps = _in_maps(query, key, value, mask, Wq, bq, Wk, bk, Wv, bv, Wo, Wd, bd)
    res = run_bass_kernel_spmd(nc, maps, core_ids=list(range(NCORES)))

    # v-projection bias folded into the output bias: x = P@(V0 + 1*bv^T)
    # normalizes to x0/Z + bv, and (x0 + bv) @ Wo + bo = x0 @ Wo + bo'
    bof = np.asarray(bv, np.float32) @ np.asarray(Wo, np.float32) + np.asarray(
        bo, np.float32
    )
    y = np.empty((B, S, D), np.float32)
    for b in range(B):
        for qh in range(2):
            c0 = b * 4 + qh * 2
            y[b, qh * SQ : (qh + 1) * SQ] = (
                res.results[c0]["yp"].astype(np.float32)
                + res.results[c0 + 1]["yp"].astype(np.float32)
                + bof[None, :]
            )
    return y



# revision 30
# speedup vs baseline: 1.2251x; 1.2251x over previous
"""Trainium2 Bass kernel for MultiHeadedAttention with learned per-key-position scaling.

Sharding over 8 NeuronCores: batch(2) x q-half(2) x head-half(2).
Each core: its batch's full keys/values, a 1024-row query slice, 6 heads.

Schedule design (cost-model-driven; PE total ~110us is the span floor, so the
schedule's one goal is a gapless tensor engine with the exp stream fed just
in time):
  - q/k path in float16 (11-bit mantissa ~ f32r precision, half the DMA bytes).
  - Host rotates kT/vT/maskT per core by its q-half offset so key chunks 0-7
    are the core's OWN q rows: the per-key-position divisor delta for those
    chunks is computed locally.  The partner half arrives via a small
    AllGather and a per-core 0/1 selector blend (SPMD-safe: selection is
    input data, the program is identical on every core).
  - The first two heads interleave in half-windows -- (h0,kc0-7), (h1,kc0-7),
    (h0,kc8-15), (h1,kc8-15) -- both run entirely on the m0 k-projection and
    the LOCAL delta half, so the partner exchange has ~16 steps of slack to
    clear the DMA queue behind the bulk stream.
  - Scores are computed transposed ([kpos, q]); delta folds into the exp's
    per-partition scale; the softmax denominator comes from a ones-column
    appended to V; P@V runs "flipped" (P stationary, V-hat streaming 65 cols).
  - All deferrable PE work (k-proj m1/m2, q-proj m2, per-head-pair V
    projections, P@V drains, pair transposes) is placed into explicit
    per-step fill tables so the tensor engine never idles: m2 re-streams kT
    during head 2 (cheaper than keeping blocks resident), V projections are
    split per head pair and re-stream vT just before each pair's drains.
  - P@V drains for a head run ~10 steps after its last score chunk; psS is a
    write-order ring (slot = step index mod 38).
  - xh->xT pair transposes run on the PE (is_transpose w/ identity) with a
    Pool-engine PSUM evacuation, keeping them off the SP DMA queue.
  - A PE warm-up spin (one long accumulation group) holds the tensor engine's
    p-state at full clock until the first projection inputs land.
  - Pool engine (otherwise idle) takes the V-hat and transpose evacuations.

The V-projection bias is folded out of the device kernel: x = P@(V0 + 1*bv^T)
normalizes to x0/Z + bv, so the host adds bv @ Wo into the output bias.

Host combines per-core partial outputs (sum over head-halves + bo').
"""

import sys

for _p in ("/opt/trn_rl_repo",):
    if _p not in sys.path:
        sys.path.insert(0, _p)

import numpy as np
import ml_dtypes

BF16 = ml_dtypes.bfloat16

B, S, D, H, DK = 2, 2048, 768, 12, 64
NCORES = 8
SQ = S // 2          # query rows per core
HH = H // 2          # heads per core
DH = HH * DK         # 384 head dims per core

_cache = {}


def _build(s=S, sq=SQ, hh=HH, d=D, dk=DK, n_qh=2, dbg=False, MASK_NG=8, SPIN=160):
    import concourse.bass as bass
    import concourse.mybir as mybir
    import concourse.tile as tile
    from concourse import bacc
    from collections import defaultdict

    f32 = mybir.dt.float32
    f16 = mybir.dt.float16
    bf = mybir.dt.bfloat16
    Exp = mybir.ActivationFunctionType.Exp
    mult = mybir.AluOpType.mult
    add = mybir.AluOpType.add
    amin = mybir.AluOpType.min
    amax = mybir.AluOpType.max

    dh = hh * dk
    KC = s // 128        # key-position chunks
    C6 = d // 128        # d_model chunks
    C3 = dh // 128       # output-dim chunks per core
    NQ = sq // 512       # 512-wide q column blocks (scores)
    QC = sq // 128       # q row chunks
    BW = 256             # streaming block width (projection inputs)
    NBK = s // BW        # key/value stream blocks
    NBQ = sq // BW       # query-slice stream blocks
    KCL = BW // 128      # kpos chunks per stream block
    PSS = 38             # psS ring slots (write-order; sized by drain lag
                         # plus one step of filler-lag margin)

    groups = [[b * 4 + hf, b * 4 + 2 + hf] for b in range(2) for hf in range(2)]
    if n_qh == 1:
        groups = None

    nc = bacc.Bacc("TRN2", target_bir_lowering=False, debug=False, num_devices=NCORES)

    t = {}
    t["qqT"] = nc.dram_tensor("qqT", [d, sq], f16, kind="ExternalInput").ap()
    t["kT"] = nc.dram_tensor("kT", [d, s], f16, kind="ExternalInput").ap()
    t["vT"] = nc.dram_tensor("vT", [d, s], bf, kind="ExternalInput").ap()
    t["maskT"] = nc.dram_tensor("maskT", [s, sq], bf, kind="ExternalInput").ap()
    t["wq"] = nc.dram_tensor("wq", [d, dh], f16, kind="ExternalInput").ap()
    t["wk"] = nc.dram_tensor("wk", [d, dh], f16, kind="ExternalInput").ap()
    t["wv"] = nc.dram_tensor("wv", [d, dh], bf, kind="ExternalInput").ap()
    t["wo"] = nc.dram_tensor("wo", [dh, d], bf, kind="ExternalInput").ap()
    t["wd"] = nc.dram_tensor("wd", [d, 1], f16, kind="ExternalInput").ap()
    t["bq"] = nc.dram_tensor("bq", [dh], f32, kind="ExternalInput").ap()
    t["bk"] = nc.dram_tensor("bk", [dh], f32, kind="ExternalInput").ap()
    t["bd"] = nc.dram_tensor("bd", [1], f32, kind="ExternalInput").ap()
    t["qsel"] = nc.dram_tensor("qsel", [2], f32, kind="ExternalInput").ap()
    t["ident"] = nc.dram_tensor("ident", [128, 128], bf, kind="ExternalInput").ap()
    t["yp"] = nc.dram_tensor("yp", [sq, d], bf, kind="ExternalOutput").ap()
    if dbg:
        t["drd"] = nc.dram_tensor("drd", [128, 16], f32, kind="ExternalOutput").ap()
        t["dxh"] = nc.dram_tensor("dxh", [128, sq // 128, hh, dk], bf, kind="ExternalOutput").ap()
        t["dxT"] = nc.dram_tensor("dxT", [128, C3, sq], bf, kind="ExternalOutput").ap()
        t["dps0"] = nc.dram_tensor("dps0", [128, 4, sq], bf, kind="ExternalOutput").ap()
        t["dkT"] = nc.dram_tensor("dkT", [128, C3, s], f16, kind="ExternalOutput").ap()
        t["dqT"] = nc.dram_tensor("dqT", [128, C3, sq], f16, kind="ExternalOutput").ap()
        t["dvsb"] = nc.dram_tensor("dvsb", [128, KC, hh, dk + 1], bf, kind="ExternalOutput").ap()
        t["dpsA"] = nc.dram_tensor("dpsA", [128, PSS, sq], bf, kind="ExternalOutput").ap()

    def dview(ap):
        return ap.rearrange("(c p) s -> p c s", p=128)

    def bcast(ap, n):
        return bass.AP(tensor=ap.tensor, offset=ap.offset, ap=[[0, n]] + list(ap.ap))

    # ---- step order: first two heads interleave in half-windows ----
    steps = (
        [(0, k) for k in range(8)] + [(1, k) for k in range(8)]
        + [(0, k) for k in range(8, 16)] + [(1, k) for k in range(8, 16)]
        + [(h, k) for h in range(2, hh) for k in range(KC)]
    )
    slot_of = {hk: i % PSS for i, hk in enumerate(steps)}

    with tile.TileContext(nc) as tc:
        with (
            tc.tile_pool(name="persist", bufs=1) as P,
            tc.tile_pool(name="pj", bufs=2, space="PSUM") as PJ,    # 2x2 banks
            tc.tile_pool(name="xpp", bufs=2, space="PSUM") as XPP,  # 2x1 bank
            tc.tile_pool(name="pp", bufs=2, space="PSUM") as PP,    # 2x1 bank
            tc.tile_pool(name="work", bufs=1) as W,
            tc.tile_pool(name="work2", bufs=2) as W2,
            tc.tile_pool(name="qload", bufs=4) as QL,
            tc.tile_pool(name="kload", bufs=4) as L,
            tc.tile_pool(name="vload", bufs=4) as LF,
            tc.tile_pool(name="dram", bufs=2, space="DRAM") as DR,
        ):
            maskT = P.tile([128, KC, sq], bf)
            vsb = P.tile([128, KC, hh, dk + 1], bf)
            psS = P.tile([128, PSS, sq], bf)
            qTh = P.tile([128, C3, sq], f16)
            kTh = P.tile([128, C3, s], f16)
            xh = P.tile([128, QC, hh, dk], bf)
            xT = P.tile([128, C3, sq], bf)
            wq_sb = P.tile([128, C6, dh], f16)
            wk_sb = P.tile([128, C6, dh], f16)
            wv_sb = P.tile([128, C6, dh], bf)
            wo_sb = P.tile([128, C3, d], bf)
            wd_sb = P.tile([128, C6, 1], f16)
            bqc = P.tile([128, C3], f32)
            bkc = P.tile([128, C3], f32)
            bdb = P.tile([128, 1], f32)
            qselb = P.tile([128, 2], f32)
            rdcol = P.tile([128, KC], f32)
            ident = P.tile([128, 128], bf)
            spinT = P.tile([128, 64], f16)

            # warm the ACT exp table while DMAs stream
            dummy = W.tile([1, 2], f32, tag="dummy")
            nc.vector.memset(dummy, 0.0)
            nc.scalar.activation(dummy, dummy, Exp, scale=1.0)

            nc.gpsimd.dma_start(wd_sb, dview(t["wd"]))
            nc.gpsimd.dma_start(bqc, t["bq"].rearrange("(c p) -> p c", p=128))
            nc.gpsimd.dma_start(bkc, t["bk"].rearrange("(c p) -> p c", p=128))
            nc.gpsimd.dma_start(bdb, bcast(t["bd"], 128))
            nc.gpsimd.dma_start(qselb, bcast(t["qsel"], 128))
            nc.gpsimd.dma_start(ident, t["ident"])
            nc.vector.memset(vsb[:, :, :, dk : dk + 1], 1.0)
            nc.vector.memset(spinT, 0.0)

            # PE warm-up spin: one long accumulation group (per-matmul side
            # effects would serialize ~9x slower than the engine time).
            sp = PP.tile([128, 512], f32, tag="pp")
            for i in range(SPIN):
                nc.tensor.matmul(
                    sp[0:64, 0:64], lhsT=spinT, rhs=spinT,
                    start=(i == 0), stop=(i == SPIN - 1),
                )

            k_tiles = {}
            v_tiles = {}

            def f16_load(src_ap, blk, pool=L, tag="ldf"):
                fr = pool.tile([128, C6, BW], f16, tag=tag)
                nc.sync.dma_start(fr, src_ap[:, :, blk * BW : (blk + 1) * BW])
                return fr

            def k_load(j):
                k_tiles[j] = f16_load(dview(t["kT"]), j % NBK)

            def v_load(b):
                vt = LF.tile([128, C6, BW], bf, tag="vb")
                nc.sync.dma_start(vt, dview(t["vT"])[:, :, b * BW : (b + 1) * BW])
                v_tiles[b] = vt

            NG = MASK_NG

            def mask_g(g):
                nc.sync.dma_start(
                    maskT[:, g * (KC // NG) : (g + 1) * (KC // NG), :],
                    t["maskT"].rearrange("(kc p) q -> p kc q", p=128)[
                        :, g * (KC // NG) : (g + 1) * (KC // NG), :
                    ],
                )

            def wv_load():
                nc.sync.dma_start(wv_sb, dview(t["wv"]))

            def wo_load():
                nc.sync.dma_start(wo_sb, t["wo"].rearrange("(c p) m -> p c m", p=128))

            def proj_m(dst, src, w_sb, bc, blk, m):
                pr = PP.tile([128, 512], f32, tag="pp")
                for c in range(C6):
                    nc.tensor.matmul(
                        pr[:, 0:BW],
                        lhsT=w_sb[:, c, m * 128 : (m + 1) * 128],
                        rhs=src[:, c, :],
                        start=(c == 0),
                        stop=(c == C6 - 1),
                    )
                nc.vector.tensor_scalar_add(
                    out=dst[:, m, blk * BW : (blk + 1) * BW],
                    in0=pr[:, 0:BW],
                    scalar1=bc[:, m : m + 1],
                )

            def v_pair(p, i):
                # two 128-kpos chunks (kcl 2i, 2i+1) of head pair p, one
                # fused DVE evacuation (GPSIMD cannot read PSUM on hw)
                vp = PP.tile([128, 512], f32, tag="pp")
                for k2 in range(2):
                    kcl = 2 * i + k2
                    for c in range(C6):
                        nc.tensor.matmul(
                            vp[:, k2 * 128 : (k2 + 1) * 128],
                            lhsT=v_tiles[kcl // KCL][:, c, (kcl % KCL) * 128 : (kcl % KCL + 1) * 128],
                            rhs=wv_sb[:, c, p * 128 : (p + 1) * 128],
                            start=(c == 0),
                            stop=(c == C6 - 1),
                        )
                nc.vector.tensor_copy(
                    vsb[:, 2 * i : 2 * i + 2, 2 * p : 2 * p + 2, 0:dk],
                    vp[:, 0:256].rearrange("p (k h e) -> p k h e", k=2, h=2),
                )

            def pv_drain(hd, qc):
                xq = XPP.tile([128, 512], f32, tag="xps")
                for kc in range(KC):
                    nc.tensor.matmul(
                        xq[:, 0 : dk + 1],
                        lhsT=psS[:, slot_of[(hd, kc)], qc * 128 : (qc + 1) * 128],
                        rhs=vsb[:, kc, hd, :],
                        start=(kc == 0),
                        stop=(kc == KC - 1),
                    )
                rz = W2.tile([128, 1], f32, tag="rz")
                nc.vector.reciprocal(rz, xq[:, dk : dk + 1])
                nc.vector.tensor_scalar_mul(
                    out=xh[:, qc, hd, :], in0=xq[:, 0:dk], scalar1=rz
                )

            def pair_tr(pair, qc):
                # xh [q, 2 heads x 64] -> xT [dims, q] on the PE + Pool evac,
                # keeping transposes off the SP DMA queue entirely
                tp = PP.tile([128, 512], bf, tag="pp")
                nc.tensor.transpose(
                    tp[:, 0:128],
                    xh[:, qc, 2 * pair : 2 * pair + 2, :].rearrange(
                        "p h e -> p (h e)"
                    ),
                    ident,
                )
                nc.vector.tensor_copy(
                    xT[:, pair, qc * 128 : (qc + 1) * 128], tp[:, 0:128]
                )

            # --- Q projection (+ local delta) over the query slice ---
            dps = XPP.tile([128, sq // 128], f32, tag="xps")
            qq_blocks = []
            for blk in range(NBQ):
                qqb = QL.tile([128, C6, BW], f16, tag="qq", name="qqb")
                eng = nc.scalar if blk == 2 else nc.sync
                eng.dma_start(qqb, dview(t["qqT"])[:, :, blk * BW : (blk + 1) * BW])
                qq_blocks.append(qqb)
                if blk == 0:
                    nc.scalar.dma_start(wq_sb, dview(t["wq"]))
                for kcl in range(KCL):
                    for c in range(C6):
                        nc.tensor.matmul(
                            dps[:, blk * KCL + kcl : blk * KCL + kcl + 1],
                            lhsT=qqb[:, c, kcl * 128 : (kcl + 1) * 128],
                            rhs=wd_sb[:, c, :],
                            start=(c == 0),
                            stop=(c == C6 - 1),
                        )
                proj_m(qTh, qqb, wq_sb, bqc, blk, 0)   # m1/m2 deferred

            k_load(0)
            nc.scalar.dma_start(wk_sb, dview(t["wk"]))

            # local delta -> recip; rdcol chunks 0-7 are purely local
            dloc = W2.tile([128, sq // 128], f32, tag="dloc")
            nc.vector.tensor_scalar(
                out=dloc, in0=dps, scalar1=bdb, scalar2=0.0, op0=add, op1=amax
            )
            nc.vector.tensor_scalar(
                out=dloc, in0=dloc, scalar1=8.0, scalar2=1.0, op0=amin, op1=add
            )
            nc.vector.reciprocal(rdcol[:, 0 : sq // 128], dloc)

            # partner half via AllGather + qsel blend; hops ride the ACT and
            # Pool queues so the SP bulk stream is never blocked
            gin = DR.tile([sq], f32)
            nc.gpsimd.dma_start(gin.rearrange("(p kcl) -> p kcl", p=128), rdcol[:, 0 : sq // 128])
            gout = DR.tile([s], f32)
            if groups is not None:
                nc.gpsimd.collective_compute(
                    "AllGather",
                    mybir.AluOpType.bypass,
                    replica_groups=groups,
                    ins=[gin.opt()],
                    outs=[gout.opt()],
                )
            else:
                nc.gpsimd.dma_start(
                    gout.rearrange("(qh rest) -> qh rest", qh=2),
                    bass.AP(
                        tensor=gin.tensor, offset=gin.offset,
                        ap=[[0, 2]] + list(gin.ap),
                    ),
                )

            gl = W2.tile([128, 2, sq // 128], f32, tag="gl")
            nc.gpsimd.dma_start(
                gl, gout.rearrange("(qh p kcl) -> p qh kcl", qh=2, p=128)
            )
            gb = W2.tile([128, 2, sq // 128], f32, tag="gb")
            nc.gpsimd.tensor_scalar_mul(
                out=gb[:, 0], in0=gl[:, 0], scalar1=qselb[:, 0:1]
            )
            nc.gpsimd.tensor_scalar_mul(
                out=gb[:, 1], in0=gl[:, 1], scalar1=qselb[:, 1:2]
            )
            nc.gpsimd.tensor_tensor(
                out=rdcol[:, sq // 128 : KC], in0=gb[:, 0], in1=gb[:, 1], op=add
            )

            # k0 m0 just in time for the first scores
            proj_m(kTh, k_tiles[0], wk_sb, bkc, 0, 0)
            with tc.tile_wait_until(0.0105):
                mask_g(0)

            # ---- placement tables (step -> work), from the cost model ----
            T0, CAD = 12.0, 1.03   # expected first-exp time / step cadence (us)

            def tw(i, lead):
                return max(0.0, T0 + i * CAD - lead) / 1000.0

            dma_at = defaultdict(list)

            def D(i, fn, lead):
                dma_at[i].append((fn, tw(i, lead)))

            for j in range(1, 8):
                D((0, 1, 3, 5, 7, 9, 11)[j - 1], lambda j=j: k_load(j), 4.0)

            D(12, wv_load, 4.0)
            for b in range(8):
                D((13, 14, 18, 19, 20, 21, 22, 23)[b], lambda b=b: v_load(b), 4.0)
            for j in range(8):
                D(25 + 2 * j, lambda j=j: k_load(NBK + j), 5.0)
            for b in range(8):
                D(41 + b, lambda b=b: v_load(b), 5.0)
            for b in range(8):
                D(56 + b, lambda b=b: v_load(b), 5.0)
            D(70, wo_load, 8.0)

            fill_at = defaultdict(list)
            pre_at = defaultdict(list)

            def F(i, fn):
                fill_at[i].append(fn)

            def FP(i, fn):
                pre_at[i].append(fn)

            # mask groups: emission must precede the first reading mask-mult
            # (tile deps are emission-ordered); the wait paces the transfer.
            mask_w = {g: tw((2, 4, 6, 8, 10, 15, 16, 17)[g], 2.0) for g in range(8)}

            def _wmask(g):
                with tc.tile_wait_until(mask_w[g]):
                    mask_g(g)

            for g in (1, 2, 3):
                FP(2 * g - 1, lambda g=g: _wmask(g))
            for g in (4, 5, 6, 7):
                FP(2 * g + 7, lambda g=g: _wmask(g))

            for j in range(1, 4):      # m0: blocks 1-3 feed (0, kc 2j)
                FP(2 * j - 1, lambda j=j: proj_m(kTh, k_tiles[j], wk_sb, bkc, j, 0))
            for j in range(4, 8):      # m0: blocks 4-7 feed (0, kc 8+)
                FP(2 * j + 1, lambda j=j: proj_m(kTh, k_tiles[j], wk_sb, bkc, j, 0))
            for j in range(4):         # k m1 before the L pool recycles blk j
                F((6, 7, 8, 10)[j], lambda j=j: proj_m(kTh, k_tiles[j], wk_sb, bkc, j, 1))
            for b in range(4):         # deferred q-proj m1 (feeds heads 2-3)
                F(12 + 2 * b, lambda b=b: proj_m(qTh, qq_blocks[b], wq_sb, bqc, b, 1))
            for j in range(4, 8):
                F(17 + 2 * (j - 4), lambda j=j: proj_m(kTh, k_tiles[j], wk_sb, bkc, j, 1))
            for i in range(8):         # pair-0 V projection, 2 chunks/step
                F(20 + i, lambda i=i: v_pair(0, i))
            for qc in range(QC):
                F(28 + qc, lambda qc=qc: pv_drain(0, qc))
            for j in range(7):         # deferred m2 from the kT re-stream
                F(33 + 2 * j, lambda j=j: proj_m(kTh, k_tiles[NBK + j], wk_sb, bkc, j, 2))
            F(49, lambda: proj_m(kTh, k_tiles[NBK + 7], wk_sb, bkc, 7, 2))
            for b, i in enumerate((36, 44, 46, 47)):   # deferred q-proj m2
                F(i, lambda b=b: proj_m(qTh, qq_blocks[b], wq_sb, bqc, b, 2))
            for qc in range(QC):
                F(37 + qc, lambda qc=qc: pv_drain(1, qc))
            for i in range(8):
                F(48 + i, lambda i=i: v_pair(1, i))
            for qc in range(QC):
                F(56 + qc, lambda qc=qc: pv_drain(2, qc))
                F(56 + qc, lambda qc=qc: pair_tr(0, qc))
            for i in range(8):
                F(64 + i, lambda i=i: v_pair(2, i))
            for qc in range(QC):
                F(72 + qc, lambda qc=qc: pv_drain(3, qc))
            for qc in range(QC):
                F(80 + qc, lambda qc=qc: pv_drain(4, qc))
            for qc in range(QC):
                F(88 + qc, lambda qc=qc: pair_tr(1, qc))


            # ---- the step loop ----
            for i, (h, kc) in enumerate(steps):
                hoff = (h % 2) * 64
                for fn in pre_at[i]:
                    fn()
                sps = PJ.tile([128, sq], f32, tag="pj")
                for nn in range(NQ):
                    nc.tensor.matmul(
                        sps[:, nn * 512 : (nn + 1) * 512],
                        lhsT=kTh[hoff : hoff + 64, h // 2, kc * 128 : (kc + 1) * 128],
                        rhs=qTh[hoff : hoff + 64, h // 2, nn * 512 : (nn + 1) * 512],
                        start=True,
                        stop=True,
                    )
                psb = psS[:, slot_of[(h, kc)], :]
                nc.scalar.activation(psb, sps, Exp, scale=rdcol[:, kc : kc + 1])
                nc.vector.tensor_tensor(out=psb, in0=psb, in1=maskT[:, kc, :], op=mult)
                for fn, w in dma_at[i]:
                    with tc.tile_wait_until(w):
                        fn()
                if i >= 1:
                    for fn in fill_at[i - 1]:
                        fn()
            for fn in fill_at[len(steps) - 1]:
                fn()

            # --- epilogue: software-pipelined last-head drains + out-proj ---
            def outproj(qc):
                yps = PJ.tile([128, d], f32, tag="pj")
                corder = [C3 - 1] + list(range(C3 - 1))
                for ci, c in enumerate(corder):
                    for col in range(0, d, 512):
                        ncol = min(512, d - col)
                        nc.tensor.matmul(
                            yps[:, col : col + ncol],
                            lhsT=xT[:, c, qc * 128 : (qc + 1) * 128],
                            rhs=wo_sb[:, c, col : col + ncol],
                            start=(ci == 0),
                            stop=(ci == C3 - 1),
                        )
                ysb = QL.tile([128, d], bf, tag="qq", name="ysb")
                if qc % 2 == 0:
                    nc.scalar.copy(ysb, yps)
                else:
                    nc.vector.tensor_copy(ysb, yps)
                nc.sync.dma_start(t["yp"][qc * 128 : (qc + 1) * 128, :], ysb)

            # two-stage lag so the PE never waits on the DVE xh-write or the
            # Pool transpose evacuation of the chunk it just produced
            for qc in range(QC):
                pv_drain(hh - 1, qc)
                if qc >= 1:
                    pair_tr(2, qc - 1)
                if qc >= 2:
                    outproj(qc - 2)
            pair_tr(2, QC - 1)
            outproj(QC - 2)
            outproj(QC - 1)
            if dbg:
                nc.sync.dma_start(t["drd"], rdcol)
                nc.sync.dma_start(t["dxh"], xh)
                nc.sync.dma_start(t["dxT"], xT)
                for ci, hk in enumerate([(0, 0), (0, 8), (5, 0), (5, 15)]):
                    nc.sync.dma_start(t["dps0"][:, ci, :], psS[:, slot_of[hk], :])
                nc.sync.dma_start(t["dkT"], kTh)
                nc.sync.dma_start(t["dqT"], qTh)
                nc.sync.dma_start(t["dvsb"], vsb)
                nc.sync.dma_start(t["dpsA"], psS)

    nc.compile()
    return nc


def _in_maps(query, key, value, mask, Wq, bq, Wk, bk, Wv, bv, Wo, Wd, bd, sq=SQ, dh=DH):
    query = np.asarray(query, np.float32)
    key = np.asarray(key, np.float32)
    value = np.asarray(value, np.float32)
    mask = np.asarray(mask)
    qT = [np.ascontiguousarray(query[b].T).astype(np.float16) for b in range(B)]
    kT = [np.ascontiguousarray(key[b].T).astype(np.float16) for b in range(B)]
    vT = [np.ascontiguousarray(value[b].T).astype(BF16) for b in range(B)]
    wqf = np.ascontiguousarray(Wq).astype(np.float16)
    wkf = np.ascontiguousarray(Wk).astype(np.float16)
    wvb = np.ascontiguousarray(Wv).astype(BF16)
    wob = np.ascontiguousarray(Wo).astype(BF16)
    wdf = np.ascontiguousarray(Wd).astype(np.float16)
    bqf = np.ascontiguousarray(bq, np.float32)
    bkf = np.ascontiguousarray(bk, np.float32)
    bdf = np.ascontiguousarray(bd, np.float32)
    identb = np.eye(128, dtype=BF16)

    maps = []
    for c in range(NCORES):
        b, qh, hf = c // 4, (c // 2) % 2, c % 2
        qs = slice(qh * sq, (qh + 1) * sq)
        hs = slice(hf * dh, (hf + 1) * dh)
        roll = -qh * sq
        kTr = np.roll(kT[b], roll, axis=1) if roll else kT[b]
        vTr = np.roll(vT[b], roll, axis=1) if roll else vT[b]
        mTr = np.ascontiguousarray(mask[b, qs].T).astype(BF16)
        if roll:
            mTr = np.roll(mTr, roll, axis=0)
        maps.append(
            {
                "qqT": np.ascontiguousarray(qT[b][:, qs]),
                "kT": np.ascontiguousarray(kTr),
                "vT": np.ascontiguousarray(vTr),
                "maskT": np.ascontiguousarray(mTr),
                "wq": np.ascontiguousarray(wqf[:, hs]),
                "wk": np.ascontiguousarray(wkf[:, hs]),
                "wv": np.ascontiguousarray(wvb[:, hs]),
                "wo": np.ascontiguousarray(wob[hs, :]),
                "wd": wdf,
                "bq": np.ascontiguousarray(bqf[hs]),
                "bk": np.ascontiguousarray(bkf[hs]),
                "bd": bdf,
                "qsel": np.array([qh, 1 - qh], np.float32),
                "ident": identb,
            }
        )
    return maps


def kernel(query, key, value, mask, Wq, bq, Wk, bk, Wv, bv, Wo, bo, Wd, bd):
    from concourse.bass_utils import run_bass_kernel_spmd

    if "nc" not in _cache:
        _cache["nc"] = _build()
    nc = _cache["nc"]

    maps = _in_maps(query, key, value, mask, Wq, bq, Wk, bk, Wv, bv, Wo, Wd, bd)
    res = run_bass_kernel_spmd(nc, maps, core_ids=list(range(NCORES)))

    # v-projection bias folded into the output bias: x = P@(V0 + 1*bv^T)
    # normalizes to x0/Z + bv, and (x0 + bv) @ Wo + bo = x0 @ Wo + bo'
    bof = np.asarray(bv, np.float32) @ np.asarray(Wo, np.float32) + np.asarray(
        bo, np.float32
    )
    y = np.empty((B, S, D), np.float32)
    for b in range(B):
        for qh in range(2):
            c0 = b * 4 + qh * 2
            y[b, qh * SQ : (qh + 1) * SQ] = (
                res.results[c0]["yp"].astype(np.float32)
                + res.results[c0 + 1]["yp"].astype(np.float32)
                + bof[None, :]
            )
    return y


# revision 31
# speedup vs baseline: 1.2287x; 1.0030x over previous
"""Trainium2 Bass kernel for MultiHeadedAttention with learned per-key-position scaling.

Sharding over 8 NeuronCores: batch(2) x q-half(2) x head-half(2).
Each core: its batch's full keys/values, a 1024-row query slice, 6 heads.

Schedule design (cost-model-driven; PE total ~110us is the span floor, so the
schedule's one goal is a gapless tensor engine with the exp stream fed just
in time):
  - q/k path in float16 (11-bit mantissa ~ f32r precision, half the DMA bytes).
  - Host rotates kT/vT/maskT per core by its q-half offset so key chunks 0-7
    are the core's OWN q rows: the per-key-position divisor delta for those
    chunks is computed locally.  The partner half arrives via a small
    AllGather and a per-core 0/1 selector blend (SPMD-safe: selection is
    input data, the program is identical on every core).
  - The first two heads interleave in half-windows -- (h0,kc0-7), (h1,kc0-7),
    (h0,kc8-15), (h1,kc8-15) -- both run entirely on the m0 k-projection and
    the LOCAL delta half, so the partner exchange has ~16 steps of slack to
    clear the DMA queue behind the bulk stream.
  - Scores are computed transposed ([kpos, q]); delta folds into the exp's
    per-partition scale; the softmax denominator comes from a ones-column
    appended to V; P@V runs "flipped" (P stationary, V-hat streaming 65 cols).
  - All deferrable PE work (k-proj m1/m2, q-proj m2, per-head-pair V
    projections, P@V drains, pair transposes) is placed into explicit
    per-step fill tables so the tensor engine never idles: m2 re-streams kT
    during head 2 (cheaper than keeping blocks resident), V projections are
    split per head pair and re-stream vT just before each pair's drains.
  - P@V drains for a head run ~10 steps after its last score chunk; psS is a
    write-order ring (slot = step index mod 38).
  - xh->xT pair transposes run on the PE (is_transpose w/ identity) with a
    Pool-engine PSUM evacuation, keeping them off the SP DMA queue.
  - A PE warm-up spin (one long accumulation group) holds the tensor engine's
    p-state at full clock until the first projection inputs land.
  - Pool engine (otherwise idle) takes the V-hat and transpose evacuations.

The V-projection bias is folded out of the device kernel: x = P@(V0 + 1*bv^T)
normalizes to x0/Z + bv, so the host adds bv @ Wo into the output bias.

Host combines per-core partial outputs (sum over head-halves + bo').
"""

import sys

for _p in ("/opt/trn_rl_repo",):
    if _p not in sys.path:
        sys.path.insert(0, _p)

import numpy as np
import ml_dtypes

BF16 = ml_dtypes.bfloat16

B, S, D, H, DK = 2, 2048, 768, 12, 64
NCORES = 8
SQ = S // 2          # query rows per core
HH = H // 2          # heads per core
DH = HH * DK         # 384 head dims per core

_cache = {}


def _build(s=S, sq=SQ, hh=HH, d=D, dk=DK, n_qh=2, dbg=False, MASK_NG=8, SPIN=160):
    import concourse.bass as bass
    import concourse.mybir as mybir
    import concourse.tile as tile
    from concourse import bacc
    from collections import defaultdict

    f32 = mybir.dt.float32
    f16 = mybir.dt.float16
    bf = mybir.dt.bfloat16
    Exp = mybir.ActivationFunctionType.Exp
    mult = mybir.AluOpType.mult
    add = mybir.AluOpType.add
    amin = mybir.AluOpType.min
    amax = mybir.AluOpType.max

    dh = hh * dk
    KC = s // 128        # key-position chunks
    C6 = d // 128        # d_model chunks
    C3 = dh // 128       # output-dim chunks per core
    NQ = sq // 512       # 512-wide q column blocks (scores)
    QC = sq // 128       # q row chunks
    BW = 256             # streaming block width (projection inputs)
    NBK = s // BW        # key/value stream blocks
    NBQ = sq // BW       # query-slice stream blocks
    KCL = BW // 128      # kpos chunks per stream block
    PSS = 38             # psS ring slots (write-order; sized by drain lag
                         # plus one step of filler-lag margin)

    groups = [[b * 4 + hf, b * 4 + 2 + hf] for b in range(2) for hf in range(2)]
    if n_qh == 1:
        groups = None

    nc = bacc.Bacc("TRN2", target_bir_lowering=False, debug=False, num_devices=NCORES)

    t = {}
    t["qqT"] = nc.dram_tensor("qqT", [d, sq], f16, kind="ExternalInput").ap()
    t["kT"] = nc.dram_tensor("kT", [d, s], f16, kind="ExternalInput").ap()
    t["vT"] = nc.dram_tensor("vT", [d, s], bf, kind="ExternalInput").ap()
    t["maskT"] = nc.dram_tensor("maskT", [s, sq], bf, kind="ExternalInput").ap()
    t["wq"] = nc.dram_tensor("wq", [d, dh], f16, kind="ExternalInput").ap()
    t["wk"] = nc.dram_tensor("wk", [d, dh], f16, kind="ExternalInput").ap()
    t["wv"] = nc.dram_tensor("wv", [d, dh], bf, kind="ExternalInput").ap()
    t["wo"] = nc.dram_tensor("wo", [dh, d], bf, kind="ExternalInput").ap()
    t["wd"] = nc.dram_tensor("wd", [d, 1], f16, kind="ExternalInput").ap()
    t["bq"] = nc.dram_tensor("bq", [dh], f32, kind="ExternalInput").ap()
    t["bk"] = nc.dram_tensor("bk", [dh], f32, kind="ExternalInput").ap()
    t["bd"] = nc.dram_tensor("bd", [1], f32, kind="ExternalInput").ap()
    t["qsel"] = nc.dram_tensor("qsel", [2], f32, kind="ExternalInput").ap()
    t["ident"] = nc.dram_tensor("ident", [128, 128], bf, kind="ExternalInput").ap()
    t["yp"] = nc.dram_tensor("yp", [sq, d], bf, kind="ExternalOutput").ap()
    if dbg:
        t["drd"] = nc.dram_tensor("drd", [128, 16], f32, kind="ExternalOutput").ap()
        t["dxh"] = nc.dram_tensor("dxh", [128, sq // 128, hh, dk], bf, kind="ExternalOutput").ap()
        t["dxT"] = nc.dram_tensor("dxT", [128, C3, sq], bf, kind="ExternalOutput").ap()
        t["dps0"] = nc.dram_tensor("dps0", [128, 4, sq], bf, kind="ExternalOutput").ap()
        t["dkT"] = nc.dram_tensor("dkT", [128, C3, s], f16, kind="ExternalOutput").ap()
        t["dqT"] = nc.dram_tensor("dqT", [128, C3, sq], f16, kind="ExternalOutput").ap()
        t["dvsb"] = nc.dram_tensor("dvsb", [128, KC, hh, dk + 1], bf, kind="ExternalOutput").ap()
        t["dpsA"] = nc.dram_tensor("dpsA", [128, PSS, sq], bf, kind="ExternalOutput").ap()

    def dview(ap):
        return ap.rearrange("(c p) s -> p c s", p=128)

    def bcast(ap, n):
        return bass.AP(tensor=ap.tensor, offset=ap.offset, ap=[[0, n]] + list(ap.ap))

    # ---- step order: first two heads interleave in half-windows ----
    steps = (
        [(0, k) for k in range(8)] + [(1, k) for k in range(8)]
        + [(0, k) for k in range(8, 16)] + [(1, k) for k in range(8, 16)]
        + [(h, k) for h in range(2, hh) for k in range(KC)]
    )
    slot_of = {hk: i % PSS for i, hk in enumerate(steps)}

    with tile.TileContext(nc) as tc:
        with (
            tc.tile_pool(name="persist", bufs=1) as P,
            tc.tile_pool(name="pj", bufs=2, space="PSUM") as PJ,    # 2x2 banks
            tc.tile_pool(name="xpp", bufs=2, space="PSUM") as XPP,  # 2x1 bank
            tc.tile_pool(name="pp", bufs=2, space="PSUM") as PP,    # 2x1 bank
            tc.tile_pool(name="work", bufs=1) as W,
            tc.tile_pool(name="work2", bufs=2) as W2,
            tc.tile_pool(name="qload", bufs=4) as QL,
            tc.tile_pool(name="kload", bufs=4) as L,
            tc.tile_pool(name="vload", bufs=4) as LF,
            tc.tile_pool(name="dram", bufs=2, space="DRAM") as DR,
        ):
            maskT = P.tile([128, KC, sq], bf)
            vsb = P.tile([128, KC, hh, dk + 1], bf)
            psS = P.tile([128, PSS, sq], bf)
            qTh = P.tile([128, C3, sq], f16)
            kTh = P.tile([128, C3, s], f16)
            xh = P.tile([128, QC, hh, dk], bf)
            xT = P.tile([128, C3, sq], bf)
            wq_sb = P.tile([128, C6, dh], f16)
            wk_sb = P.tile([128, C6, dh], f16)
            wv_sb = P.tile([128, C6, dh], bf)
            wo_sb = P.tile([128, C3, d], bf)
            wd_sb = P.tile([128, C6, 1], f16)
            bqc = P.tile([128, C3], f32)
            bkc = P.tile([128, C3], f32)
            bdb = P.tile([128, 1], f32)
            qselb = P.tile([128, 2], f32)
            rdcol = P.tile([128, KC], f32)
            ident = P.tile([128, 128], bf)
            spinT = P.tile([128, 64], f16)

            # warm the ACT exp table while DMAs stream
            dummy = W.tile([1, 2], f32, tag="dummy")
            nc.vector.memset(dummy, 0.0)
            nc.scalar.activation(dummy, dummy, Exp, scale=1.0)

            nc.gpsimd.dma_start(wd_sb, dview(t["wd"]))
            nc.gpsimd.dma_start(bqc, t["bq"].rearrange("(c p) -> p c", p=128))
            nc.gpsimd.dma_start(bkc, t["bk"].rearrange("(c p) -> p c", p=128))
            nc.gpsimd.dma_start(bdb, bcast(t["bd"], 128))
            nc.gpsimd.dma_start(qselb, bcast(t["qsel"], 128))
            nc.gpsimd.dma_start(ident, t["ident"])
            nc.vector.memset(vsb[:, :, :, dk : dk + 1], 1.0)
            nc.vector.memset(spinT, 0.0)

            # PE warm-up spin: one long accumulation group (per-matmul side
            # effects would serialize ~9x slower than the engine time).
            sp = PP.tile([128, 512], f32, tag="pp")
            for i in range(SPIN):
                nc.tensor.matmul(
                    sp[0:64, 0:64], lhsT=spinT, rhs=spinT,
                    start=(i == 0), stop=(i == SPIN - 1),
                )

            k_tiles = {}
            v_tiles = {}

            def f16_load(src_ap, blk, pool=L, tag="ldf"):
                fr = pool.tile([128, C6, BW], f16, tag=tag)
                nc.sync.dma_start(fr, src_ap[:, :, blk * BW : (blk + 1) * BW])
                return fr

            def k_load(j):
                k_tiles[j] = f16_load(dview(t["kT"]), j % NBK)

            def v_load(b):
                vt = LF.tile([128, C6, BW], bf, tag="vb")
                nc.sync.dma_start(vt, dview(t["vT"])[:, :, b * BW : (b + 1) * BW])
                v_tiles[b] = vt

            NG = MASK_NG

            def mask_g(g):
                nc.sync.dma_start(
                    maskT[:, g * (KC // NG) : (g + 1) * (KC // NG), :],
                    t["maskT"].rearrange("(kc p) q -> p kc q", p=128)[
                        :, g * (KC // NG) : (g + 1) * (KC // NG), :
                    ],
                )

            def wv_load():
                nc.sync.dma_start(wv_sb, dview(t["wv"]))

            def wo_load():
                nc.sync.dma_start(wo_sb, t["wo"].rearrange("(c p) m -> p c m", p=128))

            def proj_m(dst, src, w_sb, bc, blk, m):
                pr = PP.tile([128, 512], f32, tag="pp")
                for c in range(C6):
                    nc.tensor.matmul(
                        pr[:, 0:BW],
                        lhsT=w_sb[:, c, m * 128 : (m + 1) * 128],
                        rhs=src[:, c, :],
                        start=(c == 0),
                        stop=(c == C6 - 1),
                    )
                nc.vector.tensor_scalar_add(
                    out=dst[:, m, blk * BW : (blk + 1) * BW],
                    in0=pr[:, 0:BW],
                    scalar1=bc[:, m : m + 1],
                )

            def v_pair(p, i):
                # two 128-kpos chunks (kcl 2i, 2i+1) of head pair p, one
                # fused DVE evacuation (GPSIMD cannot read PSUM on hw)
                vp = PP.tile([128, 512], f32, tag="pp")
                for k2 in range(2):
                    kcl = 2 * i + k2
                    for c in range(C6):
                        nc.tensor.matmul(
                            vp[:, k2 * 128 : (k2 + 1) * 128],
                            lhsT=v_tiles[kcl // KCL][:, c, (kcl % KCL) * 128 : (kcl % KCL + 1) * 128],
                            rhs=wv_sb[:, c, p * 128 : (p + 1) * 128],
                            start=(c == 0),
                            stop=(c == C6 - 1),
                        )
                nc.vector.tensor_copy(
                    vsb[:, 2 * i : 2 * i + 2, 2 * p : 2 * p + 2, 0:dk],
                    vp[:, 0:256].rearrange("p (k h e) -> p k h e", k=2, h=2),
                )

            def pv_drain(hd, qc):
                xq = XPP.tile([128, 512], f32, tag="xps")
                for kc in range(KC):
                    nc.tensor.matmul(
                        xq[:, 0 : dk + 1],
                        lhsT=psS[:, slot_of[(hd, kc)], qc * 128 : (qc + 1) * 128],
                        rhs=vsb[:, kc, hd, :],
                        start=(kc == 0),
                        stop=(kc == KC - 1),
                    )
                rz = W2.tile([128, 1], f32, tag="rz")
                nc.vector.reciprocal(rz, xq[:, dk : dk + 1])
                nc.vector.tensor_scalar_mul(
                    out=xh[:, qc, hd, :], in0=xq[:, 0:dk], scalar1=rz
                )

            def pair_tr(pair, qc):
                # xh [q, 2 heads x 64] -> xT [dims, q] on the PE + Pool evac,
                # keeping transposes off the SP DMA queue entirely
                tp = PP.tile([128, 512], bf, tag="pp")
                nc.tensor.transpose(
                    tp[:, 0:128],
                    xh[:, qc, 2 * pair : 2 * pair + 2, :].rearrange(
                        "p h e -> p (h e)"
                    ),
                    ident,
                )
                nc.vector.tensor_copy(
                    xT[:, pair, qc * 128 : (qc + 1) * 128], tp[:, 0:128]
                )

            # --- Q projection (+ local delta) over the query slice ---
            dps = XPP.tile([128, sq // 128], f32, tag="xps")
            qq_blocks = []
            for blk in range(NBQ):
                qqb = QL.tile([128, C6, BW], f16, tag="qq", name="qqb")
                eng = nc.scalar if blk == 2 else nc.sync
                eng.dma_start(qqb, dview(t["qqT"])[:, :, blk * BW : (blk + 1) * BW])
                qq_blocks.append(qqb)
                if blk == 0:
                    nc.scalar.dma_start(wq_sb, dview(t["wq"]))
                for kcl in range(KCL):
                    for c in range(C6):
                        nc.tensor.matmul(
                            dps[:, blk * KCL + kcl : blk * KCL + kcl + 1],
                            lhsT=qqb[:, c, kcl * 128 : (kcl + 1) * 128],
                            rhs=wd_sb[:, c, :],
                            start=(c == 0),
                            stop=(c == C6 - 1),
                        )
                proj_m(qTh, qqb, wq_sb, bqc, blk, 0)   # m1/m2 deferred

            k_load(0)
            nc.scalar.dma_start(wk_sb, dview(t["wk"]))

            # local delta -> recip; rdcol chunks 0-7 are purely local
            dloc = W2.tile([128, sq // 128], f32, tag="dloc")
            nc.vector.tensor_scalar(
                out=dloc, in0=dps, scalar1=bdb, scalar2=0.0, op0=add, op1=amax
            )
            nc.vector.tensor_scalar(
                out=dloc, in0=dloc, scalar1=8.0, scalar2=1.0, op0=amin, op1=add
            )
            nc.vector.reciprocal(rdcol[:, 0 : sq // 128], dloc)

            # partner half via AllGather + qsel blend; hops ride the ACT and
            # Pool queues so the SP bulk stream is never blocked
            gin = DR.tile([sq], f32)
            nc.gpsimd.dma_start(gin.rearrange("(p kcl) -> p kcl", p=128), rdcol[:, 0 : sq // 128])
            gout = DR.tile([s], f32)
            if groups is not None:
                nc.gpsimd.collective_compute(
                    "AllGather",
                    mybir.AluOpType.bypass,
                    replica_groups=groups,
                    ins=[gin.opt()],
                    outs=[gout.opt()],
                )
            else:
                nc.gpsimd.dma_start(
                    gout.rearrange("(qh rest) -> qh rest", qh=2),
                    bass.AP(
                        tensor=gin.tensor, offset=gin.offset,
                        ap=[[0, 2]] + list(gin.ap),
                    ),
                )

            gl = W2.tile([128, 2, sq // 128], f32, tag="gl")
            nc.gpsimd.dma_start(
                gl, gout.rearrange("(qh p kcl) -> p qh kcl", qh=2, p=128)
            )
            gb = W2.tile([128, 2, sq // 128], f32, tag="gb")
            nc.gpsimd.tensor_scalar_mul(
                out=gb[:, 0], in0=gl[:, 0], scalar1=qselb[:, 0:1]
            )
            nc.gpsimd.tensor_scalar_mul(
                out=gb[:, 1], in0=gl[:, 1], scalar1=qselb[:, 1:2]
            )
            nc.gpsimd.tensor_tensor(
                out=rdcol[:, sq // 128 : KC], in0=gb[:, 0], in1=gb[:, 1], op=add
            )

            # k0 m0 just in time for the first scores
            proj_m(kTh, k_tiles[0], wk_sb, bkc, 0, 0)
            with tc.tile_wait_until(0.0105):
                mask_g(0)

            # ---- placement tables (step -> work), from the cost model ----
            T0, CAD = 12.0, 1.03   # expected first-exp time / step cadence (us)

            def tw(i, lead):
                return max(0.0, T0 + i * CAD - lead) / 1000.0

            dma_at = defaultdict(list)

            def D(i, fn, lead):
                dma_at[i].append((fn, tw(i, lead)))

            for j in range(1, 8):
                D((0, 1, 3, 5, 7, 9, 11)[j - 1], lambda j=j: k_load(j), 4.0)

            D(12, wv_load, 4.0)
            for b in range(8):
                D((13, 14, 18, 19, 20, 21, 22, 23)[b], lambda b=b: v_load(b), 4.0)
            for j in range(8):
                D(25 + 2 * j, lambda j=j: k_load(NBK + j), 5.0)
            for b in range(8):
                D(41 + b, lambda b=b: v_load(b), 5.0)
            for b in range(8):
                D(56 + b, lambda b=b: v_load(b), 5.0)
            D(70, wo_load, 8.0)

            fill_at = defaultdict(list)
            pre_at = defaultdict(list)

            def F(i, fn):
                fill_at[i].append(fn)

            def FP(i, fn):
                pre_at[i].append(fn)

            # mask groups: emission must precede the first reading mask-mult
            # (tile deps are emission-ordered); the wait paces the transfer.
            mask_w = {g: tw((2, 4, 6, 8, 10, 15, 16, 17)[g], 2.0) for g in range(8)}

            def _wmask(g):
                with tc.tile_wait_until(mask_w[g]):
                    mask_g(g)

            for g in (1, 2, 3):
                FP(2 * g - 1, lambda g=g: _wmask(g))
            for g in (4, 5, 6, 7):
                FP(2 * g + 7, lambda g=g: _wmask(g))

            for j in range(1, 4):      # m0: blocks 1-3 feed (0, kc 2j)
                FP(2 * j - 1, lambda j=j: proj_m(kTh, k_tiles[j], wk_sb, bkc, j, 0))
            for j in range(4, 8):      # m0: blocks 4-7 feed (0, kc 8+)
                FP(2 * j + 1, lambda j=j: proj_m(kTh, k_tiles[j], wk_sb, bkc, j, 0))
            for j in range(4):         # k m1 before the L pool recycles blk j
                F((6, 7, 8, 10)[j], lambda j=j: proj_m(kTh, k_tiles[j], wk_sb, bkc, j, 1))
            for b in range(4):         # deferred q-proj m1 (feeds heads 2-3)
                F(12 + 2 * b, lambda b=b: proj_m(qTh, qq_blocks[b], wq_sb, bqc, b, 1))
            for j in range(4, 8):
                F((13, 15, 17, 19)[j - 4], lambda j=j: proj_m(kTh, k_tiles[j], wk_sb, bkc, j, 1))
            for i in range(8):         # pair-0 V projection, 2 chunks/step
                F(20 + i, lambda i=i: v_pair(0, i))
            for qc in range(QC):
                F(28 + qc, lambda qc=qc: pv_drain(0, qc))
            for j in range(7):         # deferred m2 from the kT re-stream
                F(33 + 2 * j, lambda j=j: proj_m(kTh, k_tiles[NBK + j], wk_sb, bkc, j, 2))
            F(49, lambda: proj_m(kTh, k_tiles[NBK + 7], wk_sb, bkc, 7, 2))
            for b, i in enumerate((36, 44, 46, 47)):   # deferred q-proj m2
                F(i, lambda b=b: proj_m(qTh, qq_blocks[b], wq_sb, bqc, b, 2))
            for qc in range(QC):
                F(37 + qc, lambda qc=qc: pv_drain(1, qc))
            for i in range(8):
                F(48 + i, lambda i=i: v_pair(1, i))
            for qc in range(QC):
                F(56 + qc, lambda qc=qc: pv_drain(2, qc))
                F(56 + qc, lambda qc=qc: pair_tr(0, qc))
            for i in range(8):
                F(64 + i, lambda i=i: v_pair(2, i))
            for qc in range(QC):
                F(72 + qc, lambda qc=qc: pv_drain(3, qc))
            for qc in range(QC):
                F(80 + qc, lambda qc=qc: pv_drain(4, qc))
            for qc in range(QC):
                F(88 + qc, lambda qc=qc: pair_tr(1, qc))


            # ---- the step loop ----
            for i, (h, kc) in enumerate(steps):
                hoff = (h % 2) * 64
                for fn in pre_at[i]:
                    fn()
                sps = PJ.tile([128, sq], f32, tag="pj")
                for nn in range(NQ):
                    nc.tensor.matmul(
                        sps[:, nn * 512 : (nn + 1) * 512],
                        lhsT=kTh[hoff : hoff + 64, h // 2, kc * 128 : (kc + 1) * 128],
                        rhs=qTh[hoff : hoff + 64, h // 2, nn * 512 : (nn + 1) * 512],
                        start=True,
                        stop=True,
                    )
                psb = psS[:, slot_of[(h, kc)], :]
                nc.scalar.activation(psb, sps, Exp, scale=rdcol[:, kc : kc + 1])
                nc.vector.tensor_tensor(out=psb, in0=psb, in1=maskT[:, kc, :], op=mult)
                for fn, w in dma_at[i]:
                    with tc.tile_wait_until(w):
                        fn()
                if i >= 1:
                    for fn in fill_at[i - 1]:
                        fn()
            for fn in fill_at[len(steps) - 1]:
                fn()

            # --- epilogue: software-pipelined last-head drains + out-proj ---
            def outproj(qc):
                yps = PJ.tile([128, d], f32, tag="pj")
                corder = [C3 - 1] + list(range(C3 - 1))
                for ci, c in enumerate(corder):
                    for col in range(0, d, 512):
                        ncol = min(512, d - col)
                        nc.tensor.matmul(
                            yps[:, col : col + ncol],
                            lhsT=xT[:, c, qc * 128 : (qc + 1) * 128],
                            rhs=wo_sb[:, c, col : col + ncol],
                            start=(ci == 0),
                            stop=(ci == C3 - 1),
                        )
                ysb = QL.tile([128, d], bf, tag="qq", name="ysb")
                if qc % 2 == 0:
                    nc.scalar.copy(ysb, yps)
                else:
                    nc.vector.tensor_copy(ysb, yps)
                nc.sync.dma_start(t["yp"][qc * 128 : (qc + 1) * 128, :], ysb)

            # two-stage lag so the PE never waits on the DVE xh-write or the
            # Pool transpose evacuation of the chunk it just produced
            for qc in range(QC):
                pv_drain(hh - 1, qc)
                if qc >= 1:
                    pair_tr(2, qc - 1)
                if qc >= 2:
                    outproj(qc - 2)
            pair_tr(2, QC - 1)
            outproj(QC - 2)
            outproj(QC - 1)
            if dbg:
                nc.sync.dma_start(t["drd"], rdcol)
                nc.sync.dma_start(t["dxh"], xh)
                nc.sync.dma_start(t["dxT"], xT)
                for ci, hk in enumerate([(0, 0), (0, 8), (5, 0), (5, 15)]):
                    nc.sync.dma_start(t["dps0"][:, ci, :], psS[:, slot_of[hk], :])
                nc.sync.dma_start(t["dkT"], kTh)
                nc.sync.dma_start(t["dqT"], qTh)
                nc.sync.dma_start(t["dvsb"], vsb)
                nc.sync.dma_start(t["dpsA"], psS)

    nc.compile()
    return nc


def _in_maps(query, key, value, mask, Wq, bq, Wk, bk, Wv, bv, Wo, Wd, bd, sq=SQ, dh=DH):
    query = np.asarray(query, np.float32)
    key = np.asarray(key, np.float32)
    value = np.asarray(value, np.float32)
    mask = np.asarray(mask)
    qT = [np.ascontiguousarray(query[b].T).astype(np.float16) for b in range(B)]
    kT = [np.ascontiguousarray(key[b].T).astype(np.float16) for b in range(B)]
    vT = [np.ascontiguousarray(value[b].T).astype(BF16) for b in range(B)]
    wqf = np.ascontiguousarray(Wq).astype(np.float16)
    wkf = np.ascontiguousarray(Wk).astype(np.float16)
    wvb = np.ascontiguousarray(Wv).astype(BF16)
    wob = np.ascontiguousarray(Wo).astype(BF16)
    wdf = np.ascontiguousarray(Wd).astype(np.float16)
    bqf = np.ascontiguousarray(bq, np.float32)
    bkf = np.ascontiguousarray(bk, np.float32)
    bdf = np.ascontiguousarray(bd, np.float32)
    identb = np.eye(128, dtype=BF16)

    maps = []
    for c in range(NCORES):
        b, qh, hf = c // 4, (c // 2) % 2, c % 2
        qs = slice(qh * sq, (qh + 1) * sq)
        hs = slice(hf * dh, (hf + 1) * dh)
        roll = -qh * sq
        kTr = np.roll(kT[b], roll, axis=1) if roll else kT[b]
        vTr = np.roll(vT[b], roll, axis=1) if roll else vT[b]
        mTr = np.ascontiguousarray(mask[b, qs].T).astype(BF16)
        if roll:
            mTr = np.roll(mTr, roll, axis=0)
        maps.append(
            {
                "qqT": np.ascontiguousarray(qT[b][:, qs]),
                "kT": np.ascontiguousarray(kTr),
                "vT": np.ascontiguousarray(vTr),
                "maskT": np.ascontiguousarray(mTr),
                "wq": np.ascontiguousarray(wqf[:, hs]),
                "wk": np.ascontiguousarray(wkf[:, hs]),
                "wv": np.ascontiguousarray(wvb[:, hs]),
                "wo": np.ascontiguousarray(wob[hs, :]),
                "wd": wdf,
                "bq": np.ascontiguousarray(bqf[hs]),
                "bk": np.ascontiguousarray(bkf[hs]),
                "bd": bdf,
                "qsel": np.array([qh, 1 - qh], np.float32),
                "ident": identb,
            }
        )
    return maps


def kernel(query, key, value, mask, Wq, bq, Wk, bk, Wv, bv, Wo, bo, Wd, bd):
    from concourse.bass_utils import run_bass_kernel_spmd

    if "nc" not in _cache:
        _cache["nc"] = _build()
    nc = _cache["nc"]

    maps = _in_maps(query, key, value, mask, Wq, bq, Wk, bk, Wv, bv, Wo, Wd, bd)
    res = run_bass_kernel_spmd(nc, maps, core_ids=list(range(NCORES)))

    # v-projection bias folded into the output bias: x = P@(V0 + 1*bv^T)
    # normalizes to x0/Z + bv, and (x0 + bv) @ Wo + bo = x0 @ Wo + bo'
    bof = np.asarray(bv, np.float32) @ np.asarray(Wo, np.float32) + np.asarray(
        bo, np.float32
    )
    y = np.empty((B, S, D), np.float32)
    for b in range(B):
        for qh in range(2):
            c0 = b * 4 + qh * 2
            y[b, qh * SQ : (qh + 1) * SQ] = (
                res.results[c0]["yp"].astype(np.float32)
                + res.results[c0 + 1]["yp"].astype(np.float32)
                + bof[None, :]
            )
    return y
